# revision 1
# baseline (speedup 1.0000x reference)
"""Distributed Trainium2 Bass kernel for nn_Attention_57243324121446.

GQA attention (8 query groups, 1 kv head) with a pairwise-bias branch
(BatchRMSNorm -> exact gelu -> head projection, 4x nearest-neighbor upsample),
softclamp tanh, softmax, out-projection.

Sharding (8 cores): core c -> batch b = c//4, query groups {2*(c%4), 2*(c%4)+1}.
k/v are computed redundantly per core (single shared kv head). Pairwise is
sharded by (b, coarse-j block of 128 rows); each core computes the bias
projection for all 8 heads on its j-block, then an AllToAll within the 4-core
batch group redistributes head-pairs so each core ends with its own 2 heads
over all j. Attention outputs are produced transposed [dv, i], AllGather'ed
within the batch group into merged^T, and each core computes a disjoint
384-column slice of the final out-projection.

All matmuls run in bf16 (fp32 accumulate); softmax denominators come free via
a ones-column appended to v in the AV matmul. S is computed transposed
([j, i]) so P^T feeds the AV matmul directly with v as the stationary operand.
"""

import os
import sys

sys.path.insert(0, "/opt/trn_rl_repo")

import numpy as np
import ml_dtypes

import concourse.bass as bass
import concourse.mybir as mybir
import concourse.tile as tile
from concourse.masks import make_identity


# --- workaround: this container's walrus caps CTRL instructions at 2 sem
# waits; Tile's kernel-tail drain can carry many. Split them across drains.
def _patched_drain_and_barrier(self, tick_clock, wait_clock):
    from concourse.vector_clock import ScopedClock
    drain_inst = self.nc.sync.drain()
    wait_clock.add_sem_waits(
        drain_inst.ins, ScopedClock({None: tick_clock.global_clock})
    )
    si = drain_inst.ins.sync_info
    if si is not None and len(si.on_wait) > 1:
        waits = list(si.on_wait)
        drain_inst.ins.sync_info = mybir.SyncInfo(
            on_wait=waits[:1], on_update=list(si.on_update)
        )
        for i in range(1, len(waits)):
            extra = self.nc.sync.drain()
            extra.ins.sync_info = mybir.SyncInfo(
                on_wait=waits[i:i + 1], on_update=[]
            )
    self.nc.all_engine_barrier()
    assert self.sems is not None
    popped = self.nc._tile_sem_poison_stack.pop()
    assert popped is self._sem_poison
    self.nc.clear_and_free_semaphores(list(self.sems.allocated().values()))
    self.nc.all_engine_barrier()


tile.TileContext._drain_and_barrier = _patched_drain_and_barrier


# --- workaround 2: this walrus accepts at most ONE sem wait per instruction.
# Rewrite the BIR json before compile: hoist excess waits onto same-engine
# Nop carriers inserted immediately before the offending instruction.
import json as _json
import concourse.bass_utils as _bass_utils
import concourse.bass2jax as _bass2jax


def _split_bir_multiwaits(bir_json):
    d = _json.loads(bir_json)
    mods = d.get("modules") or [d]
    for m in mods:
        for fn in m.get("functions", []):
            for bb in fn.get("blocks", []):
                out = []
                changed = False
                for ins in bb["instructions"]:
                    si = ins.get("sync_info")
                    w = (si or {}).get("on_wait") or []
                    if len(w) > 1 and ins.get("engine"):
                        eng = ins["engine"]
                        for i, wi in enumerate(w[:-1]):
                            out.append({
                                "debug": ins.get("debug"),
                                "engine": eng,
                                "ins": [{"dtype": "int32", "kind": "imm_value",
                                         "value": 0}],
                                "name": ins["name"] + f".sw{i}",
                                "opcode": "RegisterMove",
                                "outs": [{"dtype": "int32",
                                          "kind": "register_access",
                                          "regref": f"{eng}_zero"}],
                                "sync_info": {"on_update": [], "on_wait": [wi]},
                            })
                        si["on_wait"] = [w[-1]]
                        changed = True
                    out.append(ins)
                if changed:
                    bb["instructions"] = out
    return _json.dumps(d).encode()


_orig_compile_bir = _bass_utils.compile_bir_kernel


def _patched_compile_bir(bir_json, tmpdir, neff_name="file.neff"):
    return _orig_compile_bir(_split_bir_multiwaits(bir_json), tmpdir, neff_name)


_bass_utils.compile_bir_kernel = _patched_compile_bir
_bass2jax.compile_bir_kernel = _patched_compile_bir


# --- workaround 3: the agent image's antenv lacks axon_hooks, so the boot
# shim never registers the NTFF profile hook. Provide the module and install
# the ctypes hook ourselves so run_bass_kernel_spmd(trace=True) works.
def _install_ntff_hook():
    import types as _types
    mod = sys.modules.get("antenv.axon_hooks")
    if mod is None:
        mod = _types.ModuleType("antenv.axon_hooks")
        mod._hook = None
        def _set(h):
            mod._hook = h
        def _get():
            return mod._hook
        mod.set_axon_ntff_profile_hook = _set
        mod.get_axon_ntff_profile_hook = _get
        sys.modules["antenv.axon_hooks"] = mod
        import antenv as _antenv
        _antenv.axon_hooks = mod
    if mod._hook is None and os.path.exists("/opt/axon/libaxon_pjrt.so"):
        try:
            from trn_agent_boot.trn_boot import _ntff_profile_via_ctypes
            mod._hook = _ntff_profile_via_ctypes("/opt/axon/libaxon_pjrt.so")
        except Exception as e:
            print(f"ntff hook install failed: {e}", file=sys.stderr)


_install_ntff_hook()




BF16 = mybir.dt.bfloat16
F32 = mybir.dt.float32
AF = mybir.ActivationFunctionType
ALU = mybir.AluOpType

B, N, D = 2, 2048, 1536
HEADS, KVH, DQK, DV = 8, 1, 128, 192
G = HEADS // KVH
NP, DP = 512, 128
SCALE = DQK ** -0.5
CLAMP = 5.0
MOMENTUM = 0.1
EPS = 1e-5

NCORES = 8
GPC = 2              # groups per core
JBLK = NP // 4       # coarse-j rows per core = 128
ROWS = JBLK * NP     # pairwise rows per core = 65536
RT = 512             # pairwise tile cols
NRT = ROWS // RT     # 128 pairwise tiles
TOK = 128            # token chunk
NTOK = N // TOK      # 16
DCH = D // 128       # 12 d-model chunks
JC = N // 128        # 16 j chunks
ISP = 512            # i span
NISP = N // ISP      # 4
OUTC = D // 4        # 384 out cols per core
MTOT = float(B * NP * NP)
J_ORDER = [j for j in range(JC) if j % 4 < 2] + [j for j in range(JC) if j % 4 >= 2]


def _ap(base, dims):
    return bass.AP(tensor=base.tensor, offset=base.offset, ap=dims)


def build_graph(dbg=False):
    nc = bass.Bass()

    x_T = nc.declare_dram_parameter("x_T", [128, DCH, N], BF16, isOutput=False)
    pw_T = nc.declare_dram_parameter("pw_T", [128, ROWS], BF16, isOutput=False)
    w_qkv_c = nc.declare_dram_parameter("w_qkv_c", [128, DCH, 576], BF16, isOutput=False)
    w_bias_e = nc.declare_dram_parameter("w_bias_e", [128, 8], BF16, isOutput=False)
    w_out_c = nc.declare_dram_parameter("w_out_c", [96, 16, OUTC], BF16, isOutput=False)
    b_out_c = nc.declare_dram_parameter("b_out_c", [1, OUTC], BF16, isOutput=False)
    # vecs rows: 0 qw_eff,1 qb_eff,2 kw,3 kb,4 vw(192),5 vb(192),6 gamma,
    #            7 beta,8 rv9eps
    vecs = nc.declare_dram_parameter("vecs", [12, 192], F32, isOutput=False)
    bidx = nc.declare_dram_parameter("bidx", [GPC, JC, 128], mybir.dt.int32, isOutput=False)
    eidx = nc.declare_dram_parameter("eidx", [16, 96], mybir.dt.int32, isOutput=False)
    out_c = nc.declare_dram_parameter("out_c", [N, OUTC], F32, isOutput=True)
    if dbg:
        dbg_qkT = nc.declare_dram_parameter("dbg_qkT", [128, 3 * N], F32, isOutput=True)
        dbg_ot = nc.declare_dram_parameter("dbg_ot", [2 * DV, N], F32, isOutput=True)
        dbg_v = nc.declare_dram_parameter("dbg_v", [128, NTOK * (128 + 65)], F32, isOutput=True)

    with tile.TileContext(nc) as tc:
        with tc.tile_pool(name="const", bufs=1) as const, \
             tc.tile_pool(name="dram", bufs=1, space="DRAM") as dram:

            # ---------------- constants ----------------
            ident = const.tile([128, 128], BF16)
            make_identity(nc, ident[:])
            ones1 = const.tile([1, 128], BF16)
            nc.vector.memset(ones1[:], 1.0)
            vec_sb = const.tile([128, 12], F32)
            nc.sync.dma_start(out=vec_sb[:], in_=_ap(vecs[:], [[1, 128], [192, 12]]))
            # vwb: col0 vw[0:128], col1 vb[0:128], col2 vw[128:192], col3 vb[..]
            vwb_sb = const.tile([128, 4], F32)
            nc.sync.dma_start(
                out=vwb_sb[:, 0:2], in_=_ap(vecs[4, 0], [[1, 128], [192, 2]])
            )
            nc.sync.dma_start(
                out=vwb_sb[0:64, 2:4], in_=_ap(vecs[4, 128], [[1, 64], [192, 2]])
            )
            eps_sb = const.tile([128, 1], F32)
            nc.vector.memset(eps_sb[:], EPS)
            wq_sb = const.tile([128, DCH, 576], BF16)
            nc.sync.dma_start(out=wq_sb[:], in_=w_qkv_c[:])
            wb_sb = const.tile([128, 8], BF16)
            nc.sync.dma_start(out=wb_sb[:], in_=w_bias_e[:])
            bout_sb = const.tile([128, OUTC], BF16)
            nc.vector.memset(bout_sb[:], 0.0)
            nc.sync.dma_start(out=bout_sb[:1, :], in_=b_out_c[:])
            e0row = const.tile([128, 128], BF16)
            nc.vector.memset(e0row[:], 0.0)
            nc.vector.memset(e0row[:1, :], 1.0)
            xt_sb = const.tile([128, DCH, N], BF16)
            nc.sync.dma_start(out=xt_sb[:], in_=x_T[:])
            bidx_sb = const.tile([128, GPC * JC], mybir.dt.int32)
            nc.sync.dma_start(
                out=bidx_sb[:], in_=_ap(bidx[:], [[1, 128], [128, GPC * JC]])
            )
            eidx_sb = const.tile([96, 16], mybir.dt.int32)
            nc.sync.dma_start(
                out=eidx_sb[:], in_=_ap(eidx[:], [[1, 96], [96, 16]])
            )

            qkT = const.tile([128, 3, N], BF16)       # q0^T, q1^T, k^T
            va_sb = const.tile([128, NTOK, 128], BF16)
            vb_sb = const.tile([128, NTOK, 65], BF16)
            stats = const.tile([128, NRT, 6], F32)
            mv = const.tile([128, 2], F32)
            part = const.tile([128, 2], F32)
            gsum = const.tile([128, 2], F32)
            scl = const.tile([128, 2], F32)

            var_in = dram.tile([128, 2], F32)
            var_out = dram.tile([128, 2], F32, addr_space="Shared")
            bias_own_a = dram.tile([8, ROWS // 2], BF16)
            bias_own_b = dram.tile([8, ROWS // 2], BF16)
            bias_all_a = dram.tile([8 * 8 * (JBLK // 2), NP], BF16, addr_space="Shared")
            bias_all_b = dram.tile([8 * 8 * (JBLK // 2), NP], BF16, addr_space="Shared")
            ot_own = [[dram.tile([DV, N // 2], BF16, name=f"oto{g}{k}")
                       for k in range(2)] for g in range(GPC)]
            ot_all = [[dram.tile([8 * DV, N // 2], BF16, name=f"ota{g}{k}",
                                 addr_space="Shared")
                       for k in range(2)] for g in range(GPC)]

            # ---------------- PE warmup (HAM unthrottle) ----------------
            with tc.tile_pool(name="warm", bufs=1) as warm, \
                 tc.tile_pool(name="warmps", bufs=1, space="PSUM") as warmps:
                wps = warmps.tile([128, 512], F32)
                wsb = warm.tile([128, 512], F32)
                wdr = dram.tile([128, 512], F32)
                for r in range(150):
                    nc.tensor.matmul(
                        wps[:, 0:128], ident[:], ident[:],
                        start=True, stop=True,
                    )
                nc.vector.tensor_copy(wsb[:], wps[:])
                nc.sync.dma_start(out=wdr[:], in_=wsb[:])

            # ---------------- B1: pairwise stats ----------------
            with tc.tile_pool(name="b1", bufs=3) as b1:
                for t in range(NRT // 4):
                    pt = b1.tile([128, 4, RT], BF16, tag="pwt")
                    nc.sync.dma_start(
                        out=pt[:],
                        in_=pw_T[:, t * 4 * RT:(t + 1) * 4 * RT].rearrange(
                            "p (x y) -> p x y", y=RT),
                    )
                    for q in range(4):
                        nc.vector.bn_stats(
                            out=stats[:, 4 * t + q, :], in_=pt[:, q, :]
                        )
            nc.vector.bn_aggr(out=mv[:], in_=stats[:])
            nc.vector.tensor_scalar_mul(part[:, 0:1], mv[:, 0:1], float(ROWS))
            nc.vector.tensor_tensor(
                out=part[:, 1:2], in0=mv[:, 0:1], in1=mv[:, 0:1], op=ALU.mult
            )
            nc.vector.tensor_tensor(
                out=part[:, 1:2], in0=part[:, 1:2], in1=mv[:, 1:2], op=ALU.add
            )
            nc.vector.tensor_scalar_mul(part[:, 1:2], part[:, 1:2], float(ROWS))
            nc.sync.dma_start(out=var_in[:], in_=part[:])
            nc.gpsimd.collective_compute(
                "AllReduce", ALU.add,
                replica_groups=[list(range(NCORES))],
                ins=[var_in[:].opt()], outs=[var_out[:].opt()],
            )
            nc.sync.dma_start(out=gsum[:], in_=var_out[:])

            # ---------------- C: qkv + LN + transposes ----------------
            nc.vector.memset(vb_sb[:], 0.0)
            nc.vector.memset(vb_sb[:, :, 64:65], 1.0)
            with tc.tile_pool(name="cper", bufs=1) as cper, \
                 tc.tile_pool(name="cpool", bufs=3) as cpool, \
                 tc.tile_pool(name="cps", bufs=2, space="PSUM") as cps:
                qkv_sb = cper.tile([128, NTOK, 576], BF16)
                st_all = cper.tile([128, NTOK, 4, 6], F32)
                mv_all = cper.tile([128, NTOK, 4, 2], F32)
                std_all = cper.tile([128, NTOK * 4], F32)
                for t in range(NTOK):
                    ps_qkv = cps.tile([128, 576], F32, tag="qkv")
                    for c in range(DCH):
                        nc.tensor.matmul(
                            ps_qkv[:, 0:512], xt_sb[:, c, t * TOK:(t + 1) * TOK],
                            wq_sb[:, c, 0:512], start=(c == 0),
                            stop=(c == DCH - 1),
                        )
                        nc.tensor.matmul(
                            ps_qkv[:, 512:576], xt_sb[:, c, t * TOK:(t + 1) * TOK],
                            wq_sb[:, c, 512:576], start=(c == 0),
                            stop=(c == DCH - 1),
                        )
                    nc.scalar.copy(qkv_sb[:, t, :], ps_qkv[:])
                    for s in range(4):
                        lo, hi = (s * 128, (s + 1) * 128) if s < 3 else (384, 576)
                        nc.vector.bn_stats(
                            out=st_all[:, t, s, :], in_=qkv_sb[:, t, lo:hi]
                        )
                        nc.vector.bn_aggr(
                            out=mv_all[:, t, s, :], in_=st_all[:, t, s, :]
                        )
                # rsqrt for all 64 (chunk, subrange) stats in two ops
                nc.scalar.activation(
                    out=std_all[:],
                    in_=mv_all[:].rearrange("p t s d -> p (t s d)")[:, 1::2],
                    func=AF.Sqrt, bias=eps_sb[:],
                )
                nc.vector.reciprocal(out=std_all[:], in_=std_all[:])
                nmur = cper.tile([128, NTOK * 4], F32)
                nc.vector.tensor_tensor(
                    out=nmur[:],
                    in0=mv_all[:].rearrange("p t s d -> p (t s d)")[:, 0::2],
                    in1=std_all[:], op=ALU.mult,
                )
                nc.vector.tensor_scalar_mul(nmur[:], nmur[:], -1.0)
                for t in range(NTOK):
                    nrm = cpool.tile([128, 576], BF16, tag="nrm")
                    for s in range(4):
                        lo, hi = (s * 128, (s + 1) * 128) if s < 3 else (384, 576)
                        nc.scalar.activation(
                            out=nrm[:, lo:hi], in_=qkv_sb[:, t, lo:hi],
                            func=AF.Identity,
                            scale=std_all[:, 4 * t + s:4 * t + s + 1],
                            bias=nmur[:, 4 * t + s:4 * t + s + 1],
                        )
                    for s in range(3):
                        ps_tr = cps.tile([128, 128], BF16, tag="tr")
                        nc.tensor.transpose(
                            ps_tr[:], nrm[:, s * 128:(s + 1) * 128], ident[:]
                        )
                        av = 0 if s < 2 else 2
                        nc.scalar.activation(
                            out=qkT[:, s, t * TOK:(t + 1) * TOK], in_=ps_tr[:],
                            func=AF.Identity, scale=vec_sb[:, av:av + 1],
                            bias=vec_sb[:, av + 1:av + 2],
                        )
                    nc.scalar.copy(va_sb[:, t, :], nrm[:, 384:512])
                    nc.scalar.copy(vb_sb[:, t, 0:64], nrm[:, 512:576])

            # ---------------- B-scale ----------------
            nc.vector.tensor_scalar_mul(scl[:, 0:1], gsum[:, 0:1], 1.0 / MTOT)
            nc.vector.tensor_tensor(
                out=scl[:, 0:1], in0=scl[:, 0:1], in1=scl[:, 0:1], op=ALU.mult
            )
            nc.vector.tensor_scalar_mul(scl[:, 1:2], gsum[:, 1:2], 1.0 / MTOT)
            nc.vector.tensor_tensor(
                out=scl[:, 0:1], in0=scl[:, 1:2], in1=scl[:, 0:1], op=ALU.subtract
            )
            nc.vector.tensor_scalar(
                out=scl[:, 0:1], in0=scl[:, 0:1], scalar1=MOMENTUM,
                scalar2=vec_sb[:, 8:9], op0=ALU.mult, op1=ALU.add,
            )
            nc.scalar.activation(out=scl[:, 0:1], in_=scl[:, 0:1], func=AF.Sqrt)
            nc.vector.reciprocal(out=scl[:, 0:1], in_=scl[:, 0:1])
            nc.vector.tensor_tensor(
                out=scl[:, 0:1], in0=scl[:, 0:1], in1=vec_sb[:, 6:7], op=ALU.mult
            )
            nc.vector.tensor_copy(scl[:, 1:2], vec_sb[:, 7:8])

            # ---------------- B2: gelu + bias projection ----------------
            # col-tiled: 4 pw tiles -> one [128, RT] psum (head h of tile q on
            # partition 32q+h), one DVE copy, strided DMA out.
            with tc.tile_pool(name="b2", bufs=3) as b2, \
                 tc.tile_pool(name="b2a", bufs=2) as b2a, \
                 tc.tile_pool(name="b2ps", bufs=2, space="PSUM") as b2ps:
                TB = NRT // 4   # 32 tiles of 2048 cols
                for half in range(4):
                    acc = b2a.tile([128, 8, RT], BF16, tag="acc")
                    for tl in range(8):
                        t = half * 8 + tl
                        pt = b2.tile([128, 4 * RT], BF16, tag="pwt2")
                        nc.sync.dma_start(
                            out=pt[:], in_=pw_T[:, t * 4 * RT:(t + 1) * 4 * RT]
                        )
                        gel = b2.tile([128, 4 * RT], BF16, tag="gel")
                        nc.scalar.activation(
                            out=gel[:], in_=pt[:], func=AF.Gelu,
                            bias=scl[:, 1:2], scale=scl[:, 0:1],
                        )
                        ps_b = b2ps.tile([128, RT], F32, tag="psb")
                        for q in range(4):
                            nc.tensor.matmul(
                                ps_b[32 * q:32 * q + 8, :], wb_sb[:],
                                gel[:, q * RT:(q + 1) * RT],
                                start=True, stop=True,
                                tile_position=(0, 32 * q),
                            )
                        nc.vector.tensor_copy(acc[:, tl, :], ps_b[:])
                    # dst cols within half-buffer: ((half%2)*8 + tl)*4*RT + q*RT
                    bo = bias_own_a if half < 2 else bias_own_b
                    for q in range(4):
                        nc.sync.dma_start(
                            out=_ap(
                                bo[0, (half % 2) * 8 * 4 * RT + q * RT],
                                [[ROWS // 2, 8], [4 * RT, 8], [1, RT]],
                            ),
                            in_=acc[32 * q:32 * q + 8, :, :],
                        )
                    if half == 1:
                        nc.gpsimd.collective_compute(
                            "AllGather", ALU.bypass,
                            replica_groups=[list(range(NCORES))],
                            ins=[bias_own_a[:].opt()],
                            outs=[bias_all_a[:].opt()],
                        )
            nc.gpsimd.collective_compute(
                "AllGather", ALU.bypass,
                replica_groups=[list(range(NCORES))],
                ins=[bias_own_b[:].opt()], outs=[bias_all_b[:].opt()],
            )
            # ---------------- D: attention ----------------
            HSP = N // 2
            with tc.tile_pool(name="dper", bufs=1) as dper, \
                 tc.tile_pool(name="dsb", bufs=3) as dsb, \
                 tc.tile_pool(name="osb", bufs=2) as osb, \
                 tc.tile_pool(name="dps", bufs=2, space="PSUM") as dps, \
                 tc.tile_pool(name="dpo", bufs=1, space="PSUM") as dpo:
                bias_gath = dper.tile([128, GPC, JC, NP], BF16)
                for j in J_ORDER:          # half-A chunks (j%4<2) first
                    for g in range(GPC):
                        src_t = bias_all_a if (j % 4) < 2 else bias_all_b
                        nc.gpsimd.indirect_dma_start(
                            out=bias_gath[:, g, j, :],
                            out_offset=None,
                            in_=src_t[:],
                            in_offset=bass.IndirectOffsetOnAxis(
                                ap=bidx_sb[:, g * JC + j:g * JC + j + 1], axis=0
                            ),
                        )
                for g in range(GPC):
                    for isp in range(2):
                        i0 = isp * HSP
                        ps_oa = dpo.tile([128, HSP], F32, name="psoa", tag="oa")
                        ps_ob = dpo.tile([65, HSP], F32, name="psob", tag="ob")
                        for jj, j in enumerate(J_ORDER):
                            ps_s = dps.tile([128, HSP], F32, name="pss", tag="s")
                            nc.tensor.matmul(
                                ps_s[:, 0:512],
                                qkT[:, 2, j * 128:(j + 1) * 128],
                                qkT[:, g, i0:i0 + 512],
                                start=True, stop=True,
                            )
                            nc.tensor.matmul(
                                ps_s[:, 512:1024],
                                qkT[:, 2, j * 128:(j + 1) * 128],
                                qkT[:, g, i0 + 512:i0 + 1024],
                                start=True, stop=True,
                            )
                            bt = bias_gath[:, g, j, i0 // 4:i0 // 4 + 256]
                            bb = _ap(bt, [bt.ap[0], bt.ap[1], [0, 4]])
                            u_sb = dsb.tile([128, HSP], F32, name="usb", tag="u")
                            nc.vector.tensor_tensor(
                                out=u_sb[:].rearrange("p (a b) -> p a b", b=4),
                                in0=ps_s[:].rearrange("p (a b) -> p a b", b=4),
                                in1=bb, op=ALU.add,
                            )
                            nc.scalar.activation(
                                out=u_sb[:], in_=u_sb[:], func=AF.Tanh
                            )
                            pT = dsb.tile([128, HSP], BF16, name="pTt", tag="pT")
                            nc.scalar.activation(
                                out=pT[:], in_=u_sb[:], func=AF.Exp, scale=CLAMP
                            )
                            for h in range(2):
                                nc.tensor.matmul(
                                    ps_oa[:, h * 512:(h + 1) * 512],
                                    va_sb[:, j, :], pT[:, h * 512:(h + 1) * 512],
                                    start=(jj == 0), stop=(jj == JC - 1),
                                )
                            for h in range(2):
                                nc.tensor.matmul(
                                    ps_ob[:, h * 512:(h + 1) * 512],
                                    vb_sb[:, j, :], pT[:, h * 512:(h + 1) * 512],
                                    start=(jj == 0), stop=(jj == JC - 1),
                                )
                        rd = osb.tile([1, HSP], F32, tag="rd")
                        nc.vector.reciprocal(rd[:], ps_ob[64:65, :])
                        rdb = osb.tile([1, HSP], BF16, tag="rdb")
                        nc.vector.tensor_copy(rdb[:], rd[:])
                        ps_bc = dps.tile([128, HSP], F32, name="psbc", tag="s")
                        for h in range(2):
                            nc.tensor.matmul(
                                ps_bc[:, h * 512:(h + 1) * 512], ones1[:],
                                rdb[:, h * 512:(h + 1) * 512],
                                start=True, stop=True,
                            )
                        bc_sb = osb.tile([128, HSP], BF16, tag="bc_sb")
                        nc.vector.tensor_copy(bc_sb[:], ps_bc[:])
                        oa = osb.tile([128, HSP], BF16, tag="oa_sb")
                        ob = osb.tile([64, HSP], BF16, tag="ob_sb")
                        nc.vector.tensor_tensor(
                            out=oa[:], in0=ps_oa[:], in1=bc_sb[:], op=ALU.mult
                        )
                        nc.vector.tensor_scalar(
                            out=oa[:], in0=oa[:], scalar1=vwb_sb[:, 0:1],
                            scalar2=vwb_sb[:, 1:2], op0=ALU.mult, op1=ALU.add,
                        )
                        nc.vector.tensor_tensor(
                            out=ob[:], in0=ps_ob[0:64, :], in1=bc_sb[0:64, :],
                            op=ALU.mult,
                        )
                        nc.vector.tensor_scalar(
                            out=ob[:], in0=ob[:], scalar1=vwb_sb[0:64, 2:3],
                            scalar2=vwb_sb[0:64, 3:4], op0=ALU.mult, op1=ALU.add,
                        )
                        nc.sync.dma_start(
                            out=ot_own[g][isp][0:128, :], in_=oa[:]
                        )
                        nc.sync.dma_start(
                            out=ot_own[g][isp][128:192, :], in_=ob[:]
                        )
                        nc.gpsimd.collective_compute(
                            "AllGather", ALU.bypass,
                            replica_groups=[list(range(NCORES))],
                            ins=[ot_own[g][isp][:].opt()],
                            outs=[ot_all[g][isp][:].opt()],
                        )
            # ---------------- E: out projection ----------------
            with tc.tile_pool(name="esb", bufs=3) as esb, \
                 tc.tile_pool(name="eps", bufs=2, space="PSUM") as eps_:
                wo_sb = esb.tile([96, 16, OUTC], BF16, bufs=1)
                nc.sync.dma_start(out=wo_sb[:], in_=w_out_c[:])
                mge = esb.tile([96, 2, 16, 1024], BF16, bufs=1)
                for isp in range(2):
                    for g in range(8):
                        for h in range(2):
                            nc.gpsimd.indirect_dma_start(
                                out=mge[:, isp, 2 * g + h, :],
                                out_offset=None,
                                in_=ot_all[g % 2][isp][:],
                                in_offset=bass.IndirectOffsetOnAxis(
                                    ap=eidx_sb[:, 2 * g + h:2 * g + h + 1],
                                    axis=0,
                                ),
                            )
                for t in range(NTOK):
                    isp, off = t // 8, (t % 8) * 128
                    ps_o = eps_.tile([128, OUTC], F32, tag="po")
                    for kc in range(16):
                        nc.tensor.matmul(
                            ps_o[:], mge[:, isp, kc, off:off + 128],
                            wo_sb[:, kc, :], start=(kc == 0), stop=False,
                        )
                    nc.tensor.matmul(
                        ps_o[:], e0row[:, 0:TOK], bout_sb[:],
                        start=False, stop=True,
                    )
                    o_sb = esb.tile([128, OUTC], F32, tag="o_sb")
                    nc.vector.tensor_copy(o_sb[:], ps_o[:])
                    nc.sync.dma_start(
                        out=out_c[t * TOK:(t + 1) * TOK, :], in_=o_sb[:]
                    )


    return nc


_NC_CACHE = None


def _get_nc():
    global _NC_CACHE
    if _NC_CACHE is None:
        _NC_CACHE = build_graph()
    return _NC_CACHE


def kernel(**inputs):
    from concourse.bass_utils import run_bass_kernel_spmd

    bf16 = ml_dtypes.bfloat16
    x = np.asarray(inputs["x"], np.float32)
    pairwise = np.asarray(inputs["pairwise"], np.float32)
    w_qkv = np.asarray(inputs["w_qkv"], np.float32)
    q_w = np.asarray(inputs["q_w"], np.float32)
    q_b = np.asarray(inputs["q_b"], np.float32)
    k_w = np.asarray(inputs["k_w"], np.float32)
    k_b = np.asarray(inputs["k_b"], np.float32)
    v_w = np.asarray(inputs["v_w"], np.float32)
    v_b = np.asarray(inputs["v_b"], np.float32)
    gamma = np.asarray(inputs["bias_gamma"], np.float32)
    beta = np.asarray(inputs["bias_beta"], np.float32)
    rvar = np.asarray(inputs["bias_running_var"], np.float32)
    w_bias = np.asarray(inputs["w_bias"], np.float32)
    w_out = np.asarray(inputs["w_out"], np.float32)
    b_out = np.asarray(inputs["b_out"], np.float32)

    vecs = np.zeros((12, 192), np.float32)
    vecs[0, :128] = q_w * (SCALE / CLAMP)
    vecs[1, :128] = q_b * (SCALE / CLAMP)
    vecs[2, :128] = k_w
    vecs[3, :128] = k_b
    vecs[4, :192] = v_w
    vecs[5, :192] = v_b
    vecs[6, :128] = gamma
    vecs[7, :128] = beta
    vecs[8, :128] = (1.0 - MOMENTUM) * rvar + EPS

    w_bias_e = (w_bias / CLAMP).astype(bf16)

    in_maps = []
    for c in range(NCORES):
        b, a = divmod(c, 4)
        xt = np.ascontiguousarray(
            x[b].T.reshape(DCH, 128, N).transpose(1, 0, 2)
        ).astype(bf16)
        pw = pairwise[b, :, a * JBLK:(a + 1) * JBLK, :]        # [i, jl, dp]
        pw = np.ascontiguousarray(pw.transpose(2, 1, 0).reshape(128, ROWS)
                                  ).astype(bf16)
        qcols = w_qkv[:, 2 * a * DQK:(2 * a + 2) * DQK]
        kcols = w_qkv[:, G * DQK:G * DQK + DQK]
        vcols = w_qkv[:, G * DQK + DQK:]
        wq = np.concatenate([qcols, kcols, vcols], axis=1)     # [1536, 576]
        wq = np.ascontiguousarray(
            wq.reshape(DCH, 128, 576).transpose(1, 0, 2)).astype(bf16)
        wo = w_out[:, a * OUTC:(a + 1) * OUTC]
        wo = np.ascontiguousarray(
            wo.reshape(16, 96, OUTC).transpose(1, 0, 2)).astype(bf16)
        # bias gather rows within half-buffers [8rank*8h*(JBLK/2), NP]
        gg, jj, pp = np.meshgrid(
            np.arange(GPC), np.arange(JC), np.arange(128), indexing="ij"
        )
        jl = (jj % 4) * 32 + pp // 4
        bidx_np = (
            ((b * 4 + jj // 4) * 8 + (2 * a + gg)) * (JBLK // 2)
            + np.where(jl < 64, jl, jl - 64)
        ).astype(np.int32)
        kcs, pp2 = np.meshgrid(np.arange(16), np.arange(96), indexing="ij")
        gg2, hh2 = kcs // 2, kcs % 2
        eidx_np = ((b * 4 + gg2 // 2) * 192 + 96 * hh2 + pp2).astype(np.int32)
        in_maps.append({
            "x_T": xt,
            "pw_T": pw,
            "w_qkv_c": wq,
            "w_bias_e": w_bias_e,
            "w_out_c": wo,
            "b_out_c": b_out[None, a * OUTC:(a + 1) * OUTC].astype(bf16),
            "vecs": vecs,
            "bidx": bidx_np,
            "eidx": eidx_np,
        })

    nc = _get_nc()
    res = run_bass_kernel_spmd(
        nc, in_maps, core_ids=list(range(NCORES)),
        trace=bool(int(os.environ.get("BASS_KERNEL_TRACE", "0"))),
        tmpdir=os.environ.get("BASS_KERNEL_TMPDIR"),
    )
    if res.exec_time_ns is not None:
        print(f"HW exec time: {res.exec_time_ns} ns", file=sys.stderr)

    out = np.zeros((B, N, D), np.float32)
    for c in range(NCORES):
        b, a = divmod(c, 4)
        out[b, :, a * OUTC:(a + 1) * OUTC] = res.results[c]["out_c"]
    return out

